# revision 7
# baseline (speedup 1.0000x reference)
"""GCN layer kernel for 8 Trainium2 NeuronCores.

Reference computation (N=100000 nodes, E=1600000 edges, D=64):
    msg   = (feature * norm)[src]                     # [E, D] gather
    accum = segment_sum(msg, dst, N) * norm           # [N, D] scatter-sum
    out   = accum @ W.T + b                           # [N, D]

Strategy (1D node partitioning, edges owned by dst):
  * Node space padded to 100352 = 784 windows of 128 dst nodes.
  * Windows are assigned to the 8 cores balanced by edge count (snake over
    size-sorted windows); each core owns 98 windows ("slots", sorted
    descending by size so slot k has similar cost on every core -> one SPMD
    program padded to the cross-core max chunk counts).
  * Per edge the core gathers the 256B row feature_aug[src] (64 bf16 feats +
    bf16 norm_src) from HBM with the ANT dma_gather op. Since gather indices
    are int16, the node space is split in 4 segments of 25088 rows; edges are
    grouped (batch, segment, slot) and gathered with one call per
    (batch, segment).
  * Segment-sum is done on the tensor engine: for each chunk of 128 edges an
    fp32->bf16 one-hot matrix onehot[e, m] = (dst_local[e] == m) * norm_src[e]
    is built with one DVE tensor_scalar (is_equal then mult), and
    accT[64, 128] += msgs[128, 64].T @ onehot[128, 128] accumulates in PSUM
    across all chunks/segments of the window.
  * Epilogue per window: accT -> SBUF (bf16), out = accT.T @ W.T (second
    matmul), scale rows by norm_dst (per-partition scalar), add bias, stage,
    one DMA per batch to HBM.
  * Host side only does index manipulation (sorting/padding edge ids,
    window->core assignment) and the final row un-permutation.
"""

import math
import os
from dataclasses import dataclass, field

import numpy as np
import ml_dtypes

P = 128
D = 64  # feature dim (DIN == DOUT == 64)
ROW = 128  # bf16 elements per feature_aug row (64 feats, 1 norm, 63 pad)

BF16 = ml_dtypes.bfloat16


@dataclass
class Cfg:
    n_nodes: int = 100000
    n_cores: int = 8
    seg_nodes: int = 25088  # int16 gather index limit (<32768), mult of 128
    n_seg: int = 4
    bw: int = 7   # window slots per batch
    nb: int = 14  # batches

    @property
    def npad(self):
        return self.seg_nodes * self.n_seg

    @property
    def nwin(self):
        return self.npad // P

    @property
    def wpc(self):  # windows per core
        return self.nwin // self.n_cores


@dataclass
class Plan:
    cfg: Cfg
    # static (identical across cores)
    m_cell: np.ndarray  # [wpc, n_seg] chunks per cell
    nb_bs: list  # [(b, s)] -> columns (chunks) in msgs tile
    tok_bs: list  # [(b, s)] -> tokens (=128*nb)
    ic0_bs: list  # [(b, s)] -> start column in idxs tensor (int16 cols)
    gcol_cell: np.ndarray  # [wpc, n_seg] start col in dst_all / within-run
    mcol_cell: np.ndarray  # [wpc, n_seg] start col within its (b, s) msgs tile
    ctot: int  # total chunk columns
    ictot: int  # total idx columns
    # per-core data
    in_maps: list = field(default_factory=list)
    core_slots: list = field(default_factory=list)  # [core][slot] -> window id


def make_plan(cfg: Cfg, src, dst):
    NC, WPC, NSEG, SEGN = cfg.n_cores, cfg.wpc, cfg.n_seg, cfg.seg_nodes
    assert cfg.bw * cfg.nb == WPC
    src = np.asarray(src).astype(np.int64)
    dst = np.asarray(dst).astype(np.int64)

    win = dst >> 7
    counts = np.bincount(win, minlength=cfg.nwin)
    order = np.argsort(-counts, kind="stable")
    core_slots = [[] for _ in range(NC)]
    for i, w in enumerate(order):
        r, pos = divmod(i, NC)
        c = pos if r % 2 == 0 else NC - 1 - pos
        core_slots[c].append(int(w))
    core_of = np.empty(cfg.nwin, np.int64)
    slot_of = np.empty(cfg.nwin, np.int64)
    for c in range(NC):
        for k, w in enumerate(core_slots[c]):
            core_of[w] = c
            slot_of[w] = k

    ecore = core_of[win]
    eslot = slot_of[win]
    eseg = src // SEGN
    key = ((ecore * WPC + eslot) * NSEG + eseg)
    sortidx = np.lexsort((src, key))
    skey = key[sortidx]
    ssrc = src[sortidx]
    sdst = dst[sortidx]

    ncell = NC * WPC * NSEG
    cell_start = np.searchsorted(skey, np.arange(ncell), side="left")
    cell_end = np.searchsorted(skey, np.arange(ncell), side="right")
    cnt = (cell_end - cell_start).reshape(NC, WPC, NSEG)

    m_cell = np.ceil(cnt.max(axis=0) / P).astype(np.int64)  # [WPC, NSEG]

    # static column layout in (b, s, k, c) order
    nb_bs, tok_bs, ic0_bs = {}, {}, {}
    gcol_cell = np.zeros((WPC, NSEG), np.int64)
    mcol_cell = np.zeros((WPC, NSEG), np.int64)
    gcol = 0
    icol = 0
    for b in range(cfg.nb):
        for s in range(NSEG):
            nb = 0
            for k in range(b * cfg.bw, (b + 1) * cfg.bw):
                gcol_cell[k, s] = gcol + nb
                mcol_cell[k, s] = nb
                nb += int(m_cell[k, s])
            nb_bs[(b, s)] = nb
            tok_bs[(b, s)] = nb * P
            ic0_bs[(b, s)] = icol
            icol += nb * P // 16
            gcol += nb
    ctot, ictot = gcol, icol

    plan = Plan(cfg, m_cell, nb_bs, tok_bs, ic0_bs, gcol_cell, mcol_cell,
                ctot, ictot, core_slots=core_slots)

    # per-core tensors
    for c in range(NC):
        idxs = np.zeros((P, ictot), np.int16)
        dst_all = np.full((P, ctot), -1.0, np.float32)
        for b in range(cfg.nb):
            for s in range(NSEG):
                strm_i = np.zeros(tok_bs[(b, s)], np.int16)
                strm_d = np.full(tok_bs[(b, s)], -1.0, np.float32)
                for k in range(b * cfg.bw, (b + 1) * cfg.bw):
                    w = core_slots[c][k]
                    ci = (c * WPC + k) * NSEG + s
                    e0, e1 = cell_start[ci], cell_end[ci]
                    n = e1 - e0
                    t0 = mcol_cell[k, s] * P
                    strm_i[t0:t0 + n] = (ssrc[e0:e1] - s * SEGN).astype(np.int16)
                    strm_d[t0:t0 + n] = (sdst[e0:e1] - w * P).astype(np.float32)
                ic0 = ic0_bs[(b, s)]
                nic = tok_bs[(b, s)] // 16
                if nic:
                    iw = strm_i.reshape(-1, 16).T  # [16, tok/16]
                    idxs[:, ic0:ic0 + nic] = np.tile(iw, (8, 1))
                g0 = gcol_cell[b * cfg.bw, s]
                nbv = nb_bs[(b, s)]
                if nbv:
                    dst_all[:, g0:g0 + nbv] = strm_d.reshape(nbv, P).T
        plan.in_maps.append({"idxs": idxs, "dst_all": dst_all})
    return plan


def build_program(cfg: Cfg, plan: Plan):
    from concourse import bacc, mybir
    import concourse.tile as tile

    NSEG, SEGN, BW, NB, WPC = cfg.n_seg, cfg.seg_nodes, cfg.bw, cfg.nb, cfg.wpc
    dt = mybir.dt

    nc = bacc.Bacc("TRN2", target_bir_lowering=False, debug=False,
                   num_devices=cfg.n_cores)

    feat_t = nc.dram_tensor("feature_aug", [cfg.npad, ROW], dt.bfloat16,
                            kind="ExternalInput")
    idxs_t = nc.dram_tensor("idxs", [P, plan.ictot], dt.int16,
                            kind="ExternalInput")
    dstall_t = nc.dram_tensor("dst_all", [P, plan.ctot], dt.float32,
                              kind="ExternalInput")
    normtab_t = nc.dram_tensor("norm_tab", [P, WPC], dt.float32,
                               kind="ExternalInput")
    bb_t = nc.dram_tensor("b_bcast", [P, D], dt.float32, kind="ExternalInput")
    wt_t = nc.dram_tensor("wt", [D, D], dt.bfloat16, kind="ExternalInput")
    iota_t = nc.dram_tensor("iota", [P, P], dt.bfloat16, kind="ExternalInput")
    out_t = nc.dram_tensor("out", [WPC * P, D], dt.float32,
                           kind="ExternalOutput")

    max_nb = max(plan.nb_bs.values())
    max_ic = max(plan.tok_bs[k] // 16 for k in plan.tok_bs)

    with tile.TileContext(nc) as tc:
        with (
            tc.tile_pool(name="const", bufs=1) as cpool,
            tc.tile_pool(name="msgs", bufs=8) as mpool,
            tc.tile_pool(name="idx", bufs=8) as ipool,
            tc.tile_pool(name="w", bufs=8) as wpool,
            tc.tile_pool(name="oh", bufs=6) as ohpool,
            tc.tile_pool(name="acc", bufs=6) as apool,
            tc.tile_pool(name="stage", bufs=2) as spool,
            tc.tile_pool(name="psA", bufs=2, space="PSUM") as psA,
            tc.tile_pool(name="psB", bufs=2, space="PSUM") as psB,
            tc.tile_pool(name="psO", bufs=2, space="PSUM") as psO,
        ):
            iota_s = cpool.tile([P, P], dt.bfloat16, tag="iota")
            nc.sync.dma_start(out=iota_s[:], in_=iota_t[:, :])
            wt_s = cpool.tile([D, D], dt.bfloat16, tag="wt")
            nc.sync.dma_start(out=wt_s[:], in_=wt_t[:, :])
            bb_s = cpool.tile([P, D], dt.float32, tag="bb")
            nc.sync.dma_start(out=bb_s[:], in_=bb_t[:, :])
            ntab_s = cpool.tile([P, WPC], dt.float32, tag="ntab")
            nc.sync.dma_start(out=ntab_s[:], in_=normtab_t[:, :])
            dst_s = cpool.tile([P, plan.ctot], dt.float32, tag="dst")
            nc.sync.dma_start(out=dst_s[:], in_=dstall_t[:, :])

            for b in range(NB):
                msgs = {}
                wts = {}
                for s in range(NSEG):
                    nb = plan.nb_bs[(b, s)]
                    if nb == 0:
                        continue
                    tok = plan.tok_bs[(b, s)]
                    it = ipool.tile([P, max_ic], dt.int16, tag="idx")
                    nic = tok // 16
                    nc.sync.dma_start(
                        out=it[:, :nic],
                        in_=idxs_t[:, plan.ic0_bs[(b, s)]:plan.ic0_bs[(b, s)] + nic])
                    mt = mpool.tile([P, max_nb, ROW], dt.bfloat16, tag="msgs")
                    nc.gpsimd.dma_gather(
                        mt[:, :nb, :],
                        feat_t[s * SEGN:(s + 1) * SEGN, :],
                        it[:, :nic],
                        tok,
                        tok,
                        ROW,
                        single_packet=False,
                    )
                    msgs[s] = mt
                    # norm_src per edge: cast bf16 col 64 -> f32
                    wtl = wpool.tile([P, max_nb, 1], dt.float32, tag="w")
                    nc.vector.tensor_copy(out=wtl[:, :nb, :], in_=mt[:, :nb, 64:65])
                    wts[s] = wtl

                ps_a = psA.tile([D, 4 * P], dt.float32, tag="psA")
                ps_b = psB.tile([D, max(BW - 4, 1) * P], dt.float32, tag="psB")
                stage = spool.tile([P, BW * D], dt.float32, tag="stage")

                for k7 in range(BW):
                    k = b * BW + k7
                    if k7 < 4:
                        accT = ps_a[:, k7 * P:(k7 + 1) * P]
                    else:
                        accT = ps_b[:, (k7 - 4) * P:(k7 - 3) * P]
                    chunks = [(s, c) for s in range(NSEG)
                              for c in range(int(plan.m_cell[k, s]))]
                    for ci, (s, c) in enumerate(chunks):
                        mt, wtl = msgs[s], wts[s]
                        col = int(plan.mcol_cell[k, s]) + c
                        gcol = int(plan.gcol_cell[k, s]) + c
                        oh = ohpool.tile([P, P], dt.bfloat16, tag="oh")
                        nc.vector.tensor_scalar(
                            out=oh[:],
                            in0=iota_s[:],
                            scalar1=dst_s[:, gcol:gcol + 1],
                            scalar2=wtl[:, col, :],
                            op0=mybir.AluOpType.is_equal,
                            op1=mybir.AluOpType.mult,
                        )
                        nc.tensor.matmul(
                            out=accT,
                            lhsT=mt[:, col, 0:D],
                            rhs=oh[:],
                            start=(ci == 0),
                            stop=(ci == len(chunks) - 1),
                        )
                    st_sl = stage[:, k7 * D:(k7 + 1) * D]
                    if not chunks:
                        # empty window on every core: out = bias
                        nc.vector.tensor_copy(out=st_sl, in_=bb_s[:])
                        continue
                    acc_sb = apool.tile([D, P], dt.bfloat16, tag="acc")
                    nc.vector.tensor_copy(out=acc_sb[:], in_=accT)
                    ops = psO.tile([P, D], dt.float32, tag="psO")
                    nc.tensor.matmul(out=ops[:], lhsT=acc_sb[:], rhs=wt_s[:],
                                     start=True, stop=True)
                    nc.vector.tensor_scalar(
                        out=st_sl, in0=ops[:],
                        scalar1=ntab_s[:, k:k + 1], scalar2=None,
                        op0=mybir.AluOpType.mult)
                    nc.vector.tensor_tensor(out=st_sl, in0=st_sl, in1=bb_s[:],
                                            op=mybir.AluOpType.add)
                ov = out_t[b * BW * P:(b + 1) * BW * P, :]
                ov = ov.rearrange("(kk p) d -> p kk d", p=P)
                nc.sync.dma_start(out=ov, in_=stage[:])

    nc.compile()
    return nc


def host_inputs(cfg: Cfg, plan: Plan, feature, norm, W, b):
    feature = np.asarray(feature, np.float32)
    norm = np.asarray(norm, np.float32).reshape(-1)
    n = feature.shape[0]

    feat_aug = np.zeros((cfg.npad, ROW), BF16)
    feat_aug[:n, :D] = feature.astype(BF16)
    feat_aug[:n, D] = norm.astype(BF16)

    iota = np.tile(np.arange(P, dtype=np.float32), (P, 1)).astype(BF16)
    wt = np.asarray(W, np.float32).T.astype(BF16).copy()  # [din, dout]
    b_bcast = np.tile(np.asarray(b, np.float32), (P, 1)).astype(np.float32)

    in_maps = []
    for c in range(cfg.n_cores):
        ntab = np.zeros((P, cfg.wpc), np.float32)
        for k, w in enumerate(plan.core_slots[c]):
            n0 = w * P
            n1 = min(n0 + P, n)
            if n1 > n0:
                ntab[:n1 - n0, k] = norm[n0:n1]
        m = {
            "feature_aug": feat_aug,
            "idxs": plan.in_maps[c]["idxs"],
            "dst_all": plan.in_maps[c]["dst_all"],
            "norm_tab": ntab,
            "b_bcast": b_bcast,
            "wt": wt,
            "iota": iota,
        }
        in_maps.append(m)
    return in_maps


def assemble_output(cfg: Cfg, plan: Plan, outs, n_nodes):
    full = np.zeros((n_nodes, D), np.float32)
    for c in range(cfg.n_cores):
        oc = outs[c]
        for k, w in enumerate(plan.core_slots[c]):
            n0 = w * P
            n1 = min(n0 + P, n_nodes)
            if n1 > n0:
                full[n0:n1] = oc[k * P:k * P + (n1 - n0)]
    return full


def make_runner(nc, n_cores):
    """Build the sharded jit callable around the compiled Bass program,
    mirroring bass2jax.run_bass_via_pjrt (multi-core branch)."""
    import jax
    from jax.sharding import Mesh, PartitionSpec, NamedSharding
    from jax.experimental.shard_map import shard_map
    from concourse import bass2jax, mybir

    bass2jax.install_neuronx_cc_hook()
    part_name = (nc.partition_id_tensor.name
                 if nc.partition_id_tensor is not None else None)
    in_names, out_names, out_avals, zero_outs = [], [], [], []
    for alloc in nc.m.functions[0].allocations:
        if not isinstance(alloc, mybir.MemoryLocationSet):
            continue
        name = alloc.memorylocations[0].name
        if alloc.kind == "ExternalInput":
            if name == part_name:
                continue
            in_names.append(name)
        elif alloc.kind == "ExternalOutput":
            shape = tuple(alloc.tensor_shape)
            dtype = mybir.dt.np(alloc.dtype)
            out_names.append(name)
            out_avals.append(jax.core.ShapedArray(shape, dtype))
            zero_outs.append(np.zeros(shape, dtype))
    n_params = len(in_names)

    bind_names = in_names + out_names
    if part_name is not None:
        bind_names = bind_names + [part_name]

    def _body(*args):
        operands = list(args)
        if part_name is not None:
            operands.append(bass2jax.partition_id_tensor())
        outs = bass2jax._bass_exec_p.bind(
            *operands,
            out_avals=tuple(out_avals),
            in_names=tuple(bind_names),
            out_names=tuple(out_names),
            lowering_input_output_aliases=(),
            sim_require_finite=True,
            sim_require_nnan=True,
            nc=nc,
        )
        return tuple(outs)

    devices = jax.devices()[:n_cores]
    mesh = Mesh(np.asarray(devices), ("core",))
    spec = PartitionSpec("core")
    n_outs = len(out_names)
    donate = tuple(range(n_params, n_params + n_outs))
    fn = jax.jit(
        shard_map(_body, mesh=mesh, in_specs=(spec,) * (n_params + n_outs),
                  out_specs=(spec,) * n_outs, check_rep=False),
        donate_argnums=donate, keep_unused=True)
    sharding = NamedSharding(mesh, spec)

    class Runner:
        pass

    r = Runner()
    r.fn = fn
    r.in_names = in_names
    r.out_names = out_names
    r.out_avals = out_avals
    r.zero_outs = zero_outs
    r.sharding = sharding
    r.n_cores = n_cores

    def put_inputs(in_maps):
        import jax
        concat = [np.concatenate([np.asarray(m[nm]) for m in in_maps], axis=0)
                  for nm in in_names]
        return [jax.device_put(a, sharding) for a in concat]

    def put_zeros():
        import jax
        return [jax.device_put(
            np.zeros((n_cores * z.shape[0], *z.shape[1:]), z.dtype), sharding)
            for z in zero_outs]

    def run(dev_in):
        import jax
        out = fn(*dev_in, *put_zeros())
        jax.block_until_ready(out)
        return out

    r.put_inputs = put_inputs
    r.put_zeros = put_zeros
    r.run = run
    return r


_CACHE = {}


def kernel(feature, norm, src, dst, W, b):
    cfg = Cfg()
    feature = np.asarray(feature)
    n = feature.shape[0]
    assert n == cfg.n_nodes, f"unexpected node count {n}"

    plan = make_plan(cfg, src, dst)
    key = plan.m_cell.tobytes()
    if key not in _CACHE:
        nc = build_program(cfg, plan)
        _CACHE[key] = (nc, make_runner(nc, cfg.n_cores))
    nc, runner = _CACHE[key]

    in_maps = host_inputs(cfg, plan, feature, norm, W, b)
    dev_in = runner.put_inputs(in_maps)
    out = runner.run(dev_in)
    kernel.last_runner = runner
    kernel.last_dev_in = dev_in
    oidx = runner.out_names.index("out")
    shape = runner.out_avals[oidx].shape
    arr = np.asarray(out[oidx]).reshape(cfg.n_cores, *shape)
    outs = [arr[c] for c in range(cfg.n_cores)]
    return assemble_output(cfg, plan, outs, n)


kernel.last_runner = None
kernel.last_dev_in = None


# revision 8
# speedup vs baseline: 1.2287x; 1.2287x over previous
"""GCN layer kernel for 8 Trainium2 NeuronCores.

Reference computation (N=100000 nodes, E=1600000 edges, D=64):
    msg   = (feature * norm)[src]                     # [E, D] gather
    accum = segment_sum(msg, dst, N) * norm           # [N, D] scatter-sum
    out   = accum @ W.T + b                           # [N, D]

Strategy (1D node partitioning, edges owned by dst):
  * Node space padded to 100352 = 784 windows of 128 dst nodes.
  * Windows are assigned to the 8 cores balanced by edge count (snake over
    size-sorted windows); each core owns 98 windows ("slots", sorted
    descending by size so slot k has similar cost on every core -> one SPMD
    program padded to the cross-core max chunk counts).
  * Per edge the core gathers the 256B row feature_aug[src] (64 bf16 feats +
    bf16 norm_src) from HBM with the ANT dma_gather op. Since gather indices
    are int16, the node space is split in 4 segments of 25088 rows; edges are
    grouped (batch, segment, slot) and gathered with one call per
    (batch, segment).
  * Segment-sum is done on the tensor engine: for each chunk of 128 edges an
    fp32->bf16 one-hot matrix onehot[e, m] = (dst_local[e] == m) * norm_src[e]
    is built with one DVE tensor_scalar (is_equal then mult), and
    accT[64, 128] += msgs[128, 64].T @ onehot[128, 128] accumulates in PSUM
    across all chunks/segments of the window.
  * Epilogue per window: accT -> SBUF (bf16), out = accT.T @ W.T (second
    matmul), scale rows by norm_dst (per-partition scalar), add bias, stage,
    one DMA per batch to HBM.
  * Host side only does index manipulation (sorting/padding edge ids,
    window->core assignment) and the final row un-permutation.
"""

import math
import os
from dataclasses import dataclass, field

import numpy as np
import ml_dtypes

P = 128
D = 64  # feature dim (DIN == DOUT == 64)
ROW = 128  # bf16 elements per feature_aug row (64 feats, 1 norm, 63 pad)

BF16 = ml_dtypes.bfloat16


@dataclass
class Cfg:
    n_nodes: int = 100000
    n_cores: int = 8
    seg_nodes: int = 25088  # int16 gather index limit (<32768), mult of 128
    n_seg: int = 4
    bw: int = 7   # window slots per batch
    nb: int = 14  # batches

    @property
    def npad(self):
        return self.seg_nodes * self.n_seg

    @property
    def nwin(self):
        return self.npad // P

    @property
    def wpc(self):  # windows per core
        return self.nwin // self.n_cores


@dataclass
class Plan:
    cfg: Cfg
    # static (identical across cores)
    m_cell: np.ndarray  # [wpc, n_seg] chunks per cell
    nb_bs: list  # [(b, s)] -> columns (chunks) in msgs tile
    tok_bs: list  # [(b, s)] -> tokens (=128*nb)
    ic0_bs: list  # [(b, s)] -> start column in idxs tensor (int16 cols)
    gcol_cell: np.ndarray  # [wpc, n_seg] start col in dst_all / within-run
    mcol_cell: np.ndarray  # [wpc, n_seg] start col within its (b, s) msgs tile
    ctot: int  # total chunk columns
    ictot: int  # total idx columns
    # per-core data
    in_maps: list = field(default_factory=list)
    core_slots: list = field(default_factory=list)  # [core][slot] -> window id


def make_plan(cfg: Cfg, src, dst):
    NC, WPC, NSEG, SEGN = cfg.n_cores, cfg.wpc, cfg.n_seg, cfg.seg_nodes
    assert cfg.bw * cfg.nb == WPC
    src = np.asarray(src).astype(np.int64)
    dst = np.asarray(dst).astype(np.int64)

    win = dst >> 7
    counts = np.bincount(win, minlength=cfg.nwin)
    order = np.argsort(-counts, kind="stable")
    core_slots = [[] for _ in range(NC)]
    for i, w in enumerate(order):
        r, pos = divmod(i, NC)
        c = pos if r % 2 == 0 else NC - 1 - pos
        core_slots[c].append(int(w))
    core_of = np.empty(cfg.nwin, np.int64)
    slot_of = np.empty(cfg.nwin, np.int64)
    for c in range(NC):
        for k, w in enumerate(core_slots[c]):
            core_of[w] = c
            slot_of[w] = k

    ecore = core_of[win]
    eslot = slot_of[win]
    eseg = src // SEGN
    key = ((ecore * WPC + eslot) * NSEG + eseg)
    sortidx = np.lexsort((src, key))
    skey = key[sortidx]
    ssrc = src[sortidx]
    sdst = dst[sortidx]

    ncell = NC * WPC * NSEG
    cell_start = np.searchsorted(skey, np.arange(ncell), side="left")
    cell_end = np.searchsorted(skey, np.arange(ncell), side="right")
    cnt = (cell_end - cell_start).reshape(NC, WPC, NSEG)

    m_cell = np.ceil(cnt.max(axis=0) / P).astype(np.int64)  # [WPC, NSEG]

    # static column layout in (b, s, k, c) order
    nb_bs, tok_bs, ic0_bs = {}, {}, {}
    gcol_cell = np.zeros((WPC, NSEG), np.int64)
    mcol_cell = np.zeros((WPC, NSEG), np.int64)
    gcol = 0
    icol = 0
    for b in range(cfg.nb):
        for s in range(NSEG):
            nb = 0
            for k in range(b * cfg.bw, (b + 1) * cfg.bw):
                gcol_cell[k, s] = gcol + nb
                mcol_cell[k, s] = nb
                nb += int(m_cell[k, s])
            nb_bs[(b, s)] = nb
            tok_bs[(b, s)] = nb * P
            ic0_bs[(b, s)] = icol
            icol += nb * P // 16
            gcol += nb
    ctot, ictot = gcol, icol

    plan = Plan(cfg, m_cell, nb_bs, tok_bs, ic0_bs, gcol_cell, mcol_cell,
                ctot, ictot, core_slots=core_slots)

    # per-core tensors
    for c in range(NC):
        idxs = np.zeros((P, ictot), np.int16)
        dst_all = np.full((P, ctot), -1.0, np.float32)
        for b in range(cfg.nb):
            for s in range(NSEG):
                strm_i = np.zeros(tok_bs[(b, s)], np.int16)
                strm_d = np.full(tok_bs[(b, s)], -1.0, np.float32)
                for k in range(b * cfg.bw, (b + 1) * cfg.bw):
                    w = core_slots[c][k]
                    ci = (c * WPC + k) * NSEG + s
                    e0, e1 = cell_start[ci], cell_end[ci]
                    n = e1 - e0
                    t0 = mcol_cell[k, s] * P
                    strm_i[t0:t0 + n] = (ssrc[e0:e1] - s * SEGN).astype(np.int16)
                    strm_d[t0:t0 + n] = (sdst[e0:e1] - w * P).astype(np.float32)
                ic0 = ic0_bs[(b, s)]
                nic = tok_bs[(b, s)] // 16
                if nic:
                    iw = strm_i.reshape(-1, 16).T  # [16, tok/16]
                    idxs[:, ic0:ic0 + nic] = np.tile(iw, (8, 1))
                g0 = gcol_cell[b * cfg.bw, s]
                nbv = nb_bs[(b, s)]
                if nbv:
                    dst_all[:, g0:g0 + nbv] = strm_d.reshape(nbv, P).T
        plan.in_maps.append({"idxs": idxs, "dst_all": dst_all})
    return plan


def build_program(cfg: Cfg, plan: Plan):
    from concourse import bacc, mybir
    import concourse.tile as tile

    NSEG, SEGN, BW, NB, WPC = cfg.n_seg, cfg.seg_nodes, cfg.bw, cfg.nb, cfg.wpc
    dt = mybir.dt

    nc = bacc.Bacc("TRN2", target_bir_lowering=False, debug=False,
                   num_devices=cfg.n_cores)

    feat_t = nc.dram_tensor("feature_aug", [cfg.npad, ROW], dt.bfloat16,
                            kind="ExternalInput")
    idxs_t = nc.dram_tensor("idxs", [P, plan.ictot], dt.int16,
                            kind="ExternalInput")
    dstall_t = nc.dram_tensor("dst_all", [P, plan.ctot], dt.float32,
                              kind="ExternalInput")
    normtab_t = nc.dram_tensor("norm_tab", [P, WPC], dt.float32,
                               kind="ExternalInput")
    bb_t = nc.dram_tensor("b_bcast", [P, D], dt.float32, kind="ExternalInput")
    wt_t = nc.dram_tensor("wt", [D, D], dt.bfloat16, kind="ExternalInput")
    iota_t = nc.dram_tensor("iota", [P, P], dt.bfloat16, kind="ExternalInput")
    out_t = nc.dram_tensor("out", [WPC * P, D], dt.float32,
                           kind="ExternalOutput")

    max_nb = max(plan.nb_bs.values())
    max_ic = max(plan.tok_bs[k] // 16 for k in plan.tok_bs)

    with tile.TileContext(nc) as tc:
        with (
            tc.tile_pool(name="const", bufs=1) as cpool,
            tc.tile_pool(name="msgs", bufs=8) as mpool,
            tc.tile_pool(name="idx", bufs=8) as ipool,
            tc.tile_pool(name="w", bufs=8) as wpool,
            tc.tile_pool(name="oh", bufs=6) as ohpool,
            tc.tile_pool(name="acc", bufs=6) as apool,
            tc.tile_pool(name="stage", bufs=2) as spool,
            tc.tile_pool(name="psA", bufs=2, space="PSUM") as psA,
            tc.tile_pool(name="psB", bufs=2, space="PSUM") as psB,
            tc.tile_pool(name="psO", bufs=2, space="PSUM") as psO,
        ):
            iota_s = cpool.tile([P, P], dt.bfloat16, tag="iota")
            nc.sync.dma_start(out=iota_s[:], in_=iota_t[:, :])
            wt_s = cpool.tile([D, D], dt.bfloat16, tag="wt")
            nc.sync.dma_start(out=wt_s[:], in_=wt_t[:, :])
            bb_s = cpool.tile([P, D], dt.float32, tag="bb")
            nc.sync.dma_start(out=bb_s[:], in_=bb_t[:, :])
            ntab_s = cpool.tile([P, WPC], dt.float32, tag="ntab")
            nc.sync.dma_start(out=ntab_s[:], in_=normtab_t[:, :])
            dst_s = cpool.tile([P, plan.ctot], dt.float32, tag="dst")
            nc.sync.dma_start(out=dst_s[:], in_=dstall_t[:, :])

            for b in range(NB):
                msgs = {}
                wts = {}
                for s in range(NSEG):
                    nb = plan.nb_bs[(b, s)]
                    if nb == 0:
                        continue
                    tok = plan.tok_bs[(b, s)]
                    it = ipool.tile([P, max_ic], dt.int16, tag="idx")
                    nic = tok // 16
                    nc.sync.dma_start(
                        out=it[:, :nic],
                        in_=idxs_t[:, plan.ic0_bs[(b, s)]:plan.ic0_bs[(b, s)] + nic])
                    mt = mpool.tile([P, max_nb, ROW], dt.bfloat16, tag="msgs")
                    nc.gpsimd.dma_gather(
                        mt[:, :nb, :],
                        feat_t[s * SEGN:(s + 1) * SEGN, :],
                        it[:, :nic],
                        tok,
                        tok,
                        ROW,
                        single_packet=False,
                    )
                    msgs[s] = mt
                    # norm_src per edge: cast bf16 col 64 -> f32
                    wtl = wpool.tile([P, max_nb, 1], dt.float32, tag="w")
                    nc.vector.tensor_copy(out=wtl[:, :nb, :], in_=mt[:, :nb, 64:65])
                    wts[s] = wtl

                ps_a = psA.tile([D, 4 * P], dt.float32, tag="psA")
                ps_b = psB.tile([D, max(BW - 4, 1) * P], dt.float32, tag="psB")
                stage = spool.tile([P, BW * D], dt.float32, tag="stage")

                for k7 in range(BW):
                    k = b * BW + k7
                    if k7 < 4:
                        accT = ps_a[:, k7 * P:(k7 + 1) * P]
                    else:
                        accT = ps_b[:, (k7 - 4) * P:(k7 - 3) * P]
                    chunks = [(s, c) for s in range(NSEG)
                              for c in range(int(plan.m_cell[k, s]))]
                    for ci, (s, c) in enumerate(chunks):
                        mt, wtl = msgs[s], wts[s]
                        col = int(plan.mcol_cell[k, s]) + c
                        gcol = int(plan.gcol_cell[k, s]) + c
                        oh = ohpool.tile([P, P], dt.bfloat16, tag="oh")
                        nc.vector.tensor_scalar(
                            out=oh[:],
                            in0=iota_s[:],
                            scalar1=dst_s[:, gcol:gcol + 1],
                            scalar2=wtl[:, col, :],
                            op0=mybir.AluOpType.is_equal,
                            op1=mybir.AluOpType.mult,
                        )
                        nc.tensor.matmul(
                            out=accT,
                            lhsT=mt[:, col, 0:D],
                            rhs=oh[:],
                            start=(ci == 0),
                            stop=(ci == len(chunks) - 1),
                        )
                    st_sl = stage[:, k7 * D:(k7 + 1) * D]
                    if not chunks:
                        # empty window on every core: out = bias
                        nc.vector.tensor_copy(out=st_sl, in_=bb_s[:])
                        continue
                    acc_sb = apool.tile([D, P], dt.bfloat16, tag="acc")
                    nc.vector.tensor_copy(out=acc_sb[:], in_=accT)
                    ops = psO.tile([P, D], dt.float32, tag="psO")
                    nc.tensor.matmul(out=ops[:], lhsT=acc_sb[:], rhs=wt_s[:],
                                     start=True, stop=True)
                    nc.vector.tensor_scalar(
                        out=st_sl, in0=ops[:],
                        scalar1=ntab_s[:, k:k + 1], scalar2=None,
                        op0=mybir.AluOpType.mult)
                    nc.vector.tensor_tensor(out=st_sl, in0=st_sl, in1=bb_s[:],
                                            op=mybir.AluOpType.add)
                ov = out_t[b * BW * P:(b + 1) * BW * P, :]
                ov = ov.rearrange("(kk p) d -> p kk d", p=P)
                nc.sync.dma_start(out=ov, in_=stage[:])

    nc.compile()
    return nc


def host_inputs(cfg: Cfg, plan: Plan, feature, norm, W, b):
    feature = np.asarray(feature, np.float32)
    norm = np.asarray(norm, np.float32).reshape(-1)
    n = feature.shape[0]

    feat_aug = np.zeros((cfg.npad, ROW), BF16)
    feat_aug[:n, :D] = feature.astype(BF16)
    feat_aug[:n, D] = norm.astype(BF16)

    iota = np.tile(np.arange(P, dtype=np.float32), (P, 1)).astype(BF16)
    wt = np.asarray(W, np.float32).T.astype(BF16).copy()  # [din, dout]
    b_bcast = np.tile(np.asarray(b, np.float32), (P, 1)).astype(np.float32)

    in_maps = []
    for c in range(cfg.n_cores):
        ntab = np.zeros((P, cfg.wpc), np.float32)
        for k, w in enumerate(plan.core_slots[c]):
            n0 = w * P
            n1 = min(n0 + P, n)
            if n1 > n0:
                ntab[:n1 - n0, k] = norm[n0:n1]
        m = {
            "feature_aug": feat_aug,
            "idxs": plan.in_maps[c]["idxs"],
            "dst_all": plan.in_maps[c]["dst_all"],
            "norm_tab": ntab,
            "b_bcast": b_bcast,
            "wt": wt,
            "iota": iota,
        }
        in_maps.append(m)
    return in_maps


def assemble_output(cfg: Cfg, plan: Plan, outs, n_nodes):
    full = np.zeros((n_nodes, D), np.float32)
    for c in range(cfg.n_cores):
        oc = outs[c]
        for k, w in enumerate(plan.core_slots[c]):
            n0 = w * P
            n1 = min(n0 + P, n_nodes)
            if n1 > n0:
                full[n0:n1] = oc[k * P:k * P + (n1 - n0)]
    return full


def make_runner(nc, n_cores):
    """Build the sharded jit callable around the compiled Bass program,
    mirroring bass2jax.run_bass_via_pjrt (multi-core branch)."""
    import jax
    from jax.sharding import Mesh, PartitionSpec, NamedSharding
    from jax.experimental.shard_map import shard_map
    from concourse import bass2jax, mybir

    bass2jax.install_neuronx_cc_hook()
    part_name = (nc.partition_id_tensor.name
                 if nc.partition_id_tensor is not None else None)
    in_names, out_names, out_avals, zero_outs = [], [], [], []
    for alloc in nc.m.functions[0].allocations:
        if not isinstance(alloc, mybir.MemoryLocationSet):
            continue
        name = alloc.memorylocations[0].name
        if alloc.kind == "ExternalInput":
            if name == part_name:
                continue
            in_names.append(name)
        elif alloc.kind == "ExternalOutput":
            shape = tuple(alloc.tensor_shape)
            dtype = mybir.dt.np(alloc.dtype)
            out_names.append(name)
            out_avals.append(jax.core.ShapedArray(shape, dtype))
            zero_outs.append(np.zeros(shape, dtype))
    n_params = len(in_names)

    bind_names = in_names + out_names
    if part_name is not None:
        bind_names = bind_names + [part_name]

    def _body(*args):
        operands = list(args)
        if part_name is not None:
            operands.append(bass2jax.partition_id_tensor())
        outs = bass2jax._bass_exec_p.bind(
            *operands,
            out_avals=tuple(out_avals),
            in_names=tuple(bind_names),
            out_names=tuple(out_names),
            lowering_input_output_aliases=(),
            sim_require_finite=True,
            sim_require_nnan=True,
            nc=nc,
        )
        return tuple(outs)

    devices = jax.devices()[:n_cores]
    mesh = Mesh(np.asarray(devices), ("core",))
    spec = PartitionSpec("core")
    n_outs = len(out_names)
    donate = tuple(range(n_params, n_params + n_outs))
    fn = jax.jit(
        shard_map(_body, mesh=mesh, in_specs=(spec,) * (n_params + n_outs),
                  out_specs=(spec,) * n_outs, check_rep=False),
        donate_argnums=donate, keep_unused=True)
    sharding = NamedSharding(mesh, spec)

    class Runner:
        pass

    r = Runner()
    r.fn = fn
    r.in_names = in_names
    r.out_names = out_names
    r.out_avals = out_avals
    r.zero_outs = zero_outs
    r.sharding = sharding
    r.n_cores = n_cores

    def put_inputs(in_maps):
        import jax
        concat = [np.concatenate([np.asarray(m[nm]) for m in in_maps], axis=0)
                  for nm in in_names]
        return [jax.device_put(a, sharding) for a in concat]

    def put_zeros():
        import jax
        return [jax.device_put(
            np.zeros((n_cores * z.shape[0], *z.shape[1:]), z.dtype), sharding)
            for z in zero_outs]

    def run(dev_in, dev_zeros=None):
        import jax
        if dev_zeros is None:
            dev_zeros = put_zeros()
        out = fn(*dev_in, *dev_zeros)
        jax.block_until_ready(out)
        return out

    r.put_inputs = put_inputs
    r.put_zeros = put_zeros
    r.run = run
    return r


_CACHE = {}


def kernel(feature, norm, src, dst, W, b):
    cfg = Cfg()
    feature = np.asarray(feature)
    n = feature.shape[0]
    assert n == cfg.n_nodes, f"unexpected node count {n}"

    plan = make_plan(cfg, src, dst)
    key = plan.m_cell.tobytes()
    if key not in _CACHE:
        nc = build_program(cfg, plan)
        _CACHE[key] = (nc, make_runner(nc, cfg.n_cores))
    nc, runner = _CACHE[key]

    in_maps = host_inputs(cfg, plan, feature, norm, W, b)
    dev_in = runner.put_inputs(in_maps)
    out = runner.run(dev_in)
    kernel.last_runner = runner
    kernel.last_dev_in = dev_in
    oidx = runner.out_names.index("out")
    shape = runner.out_avals[oidx].shape
    arr = np.asarray(out[oidx]).reshape(cfg.n_cores, *shape)
    outs = [arr[c] for c in range(cfg.n_cores)]
    return assemble_output(cfg, plan, outs, n)


kernel.last_runner = None
kernel.last_dev_in = None


# revision 24
# speedup vs baseline: 1.6000x; 1.3022x over previous
"""GCN layer kernel for 8 Trainium2 NeuronCores.

Reference computation (N=100000 nodes, E=1600000 edges, D=64):
    msg   = (feature * norm)[src]                     # [E, D] gather
    accum = segment_sum(msg, dst, N) * norm           # [N, D] scatter-sum
    out   = accum @ W.T + b                           # [N, D]

Strategy (1D node partitioning, edges owned by dst):
  * Node space padded to 100352 = 784 windows of 128 dst nodes.
  * Windows are assigned to the 8 cores balanced by edge count (snake over
    size-sorted windows); each core owns 98 windows ("slots"), processed in
    14 batches of 7.  Slot k is the k-th largest window on every core, so one
    SPMD program padded to the cross-core max cell sizes fits all cores.
  * Per edge the core gathers the 256B row feature_aug[src] (64 bf16 feats +
    bf16 norm_src in col 64) from HBM with the ANT dma_gather op.  Gather
    indices are int16, so the node space is covered by 4 overlapping gather
    segments (base B_s = max(0, s*25088 - ov)); edges pick the segment whose
    window keeps their index in [0, 32768), with split points balancing the
    4 cells of each window.
  * Segment-sum runs on the tensor engine: for each chunk of <=128 edges a
    bf16 one-hot onehot[e, m] = (dst_local[e] == m) * norm_src[e] is built
    with one DVE tensor_scalar (is_equal, mult), and
    accT[64, 128] += msgs[:, :64].T @ onehot accumulates in PSUM across all
    chunks/segments of the window.
  * Epilogue per window: accT -> SBUF bf16, out = accT.T @ W.T (2nd matmul),
    rows scaled by norm_dst (per-partition scalar), plus bias, staged and
    written back once per batch.
  * Host side only does index manipulation (sorting/padding edge ids,
    window->core assignment) and the final row un-permutation.
"""

import os
from dataclasses import dataclass, field

import numpy as np
import ml_dtypes

P = 128
D = 64  # feature dim (DIN == DOUT == 64)
ROW = 128  # bf16 elements per feature_aug row (64 feats, 1 norm, 63 pad)

BF16 = ml_dtypes.bfloat16


@dataclass
class Cfg:
    n_nodes: int = 100000
    n_cores: int = 8
    seg_nodes: int = 25088  # int16 gather index limit (<32768), mult of 128
    n_seg: int = 4
    bw: int = 7   # window slots per batch
    nb: int = 14  # batches
    gran: int = 32  # cell size granularity (32/64/128); <128 uses PE row tiles
    ov: int = 3840  # segment overlap for cell balancing (0 = exact segments)

    @property
    def npad(self):
        return self.seg_nodes * self.n_seg

    @property
    def nwin(self):
        return self.npad // P

    @property
    def wpc(self):  # windows per core
        return self.nwin // self.n_cores


@dataclass
class Plan:
    cfg: Cfg
    # static (identical across cores)
    t_cell: np.ndarray  # [wpc, n_seg] tokens per cell (gran-mult)
    nb_bs: dict  # (b, s) -> msgs tile columns
    tok_bs: dict  # (b, s) -> tokens (= 128*nb)
    ic0_bs: dict  # (b, s) -> start col in int16 idxs tensor
    gcol0_bs: dict  # (b, s) -> start col in dst_all
    cell_t0: np.ndarray  # [wpc, n_seg] token offset of cell within its call
    chunk_table: dict  # b -> {k7: [(s, col, p0, kc), ...]}
    ctot: int
    ictot: int
    # per-core data
    in_maps: list = field(default_factory=list)
    core_slots: list = field(default_factory=list)  # [core][slot] -> window id


def _seg_base(cfg, s):
    return max(0, s * cfg.seg_nodes - cfg.ov)


def make_plan(cfg: Cfg, src, dst):
    NC, WPC, NSEG, SEGN = cfg.n_cores, cfg.wpc, cfg.n_seg, cfg.seg_nodes
    assert cfg.bw * cfg.nb == WPC
    src = np.asarray(src).astype(np.int64)
    dst = np.asarray(dst).astype(np.int64)

    win = dst >> 7
    counts = np.bincount(win, minlength=cfg.nwin)
    order = np.argsort(-counts, kind="stable")
    core_slots = [[] for _ in range(NC)]
    for i, w in enumerate(order):
        r, pos = divmod(i, NC)
        c = pos if r % 2 == 0 else NC - 1 - pos
        core_slots[c].append(int(w))
    core_of = np.empty(cfg.nwin, np.int64)
    slot_of = np.empty(cfg.nwin, np.int64)
    for c in range(NC):
        for k, w in enumerate(core_slots[c]):
            core_of[w] = c
            slot_of[w] = k

    key = core_of[win] * WPC + slot_of[win]
    sortidx = np.lexsort((src, key))
    skey = key[sortidx]
    ssrc = src[sortidx]
    sdst = dst[sortidx]

    ncell = NC * WPC
    w_start = np.searchsorted(skey, np.arange(ncell), side="left")
    w_end = np.searchsorted(skey, np.arange(ncell), side="right")

    # Per (core, slot): 3 split points in the src-sorted edge list, each
    # inside the +-ov zone around its segment boundary, targeting quarters.
    splits = np.zeros((NC, WPC, NSEG + 1), np.int64)
    for c in range(NC):
        for k in range(WPC):
            ci = c * WPC + k
            e0, e1 = w_start[ci], w_end[ci]
            srcs = ssrc[e0:e1]
            n = e1 - e0
            sp = [0]
            for s in range(1, NSEG):
                bnd = s * SEGN
                lo = np.searchsorted(srcs, bnd - cfg.ov)
                hi = np.searchsorted(srcs, bnd + cfg.ov)
                tgt = int(round(n * s / NSEG))
                sp.append(int(min(max(tgt, lo, sp[-1]), hi)))
            sp.append(n)
            splits[c, k] = sp

    cell_cnt = np.diff(splits, axis=2)  # [NC, WPC, NSEG]
    g = cfg.gran
    t_cell = (np.ceil(cell_cnt.max(axis=0) / g) * g).astype(np.int64)

    nb_bs, tok_bs, ic0_bs, gcol0_bs = {}, {}, {}, {}
    cell_t0 = np.zeros((WPC, NSEG), np.int64)
    chunk_table = {}
    gcol = 0
    icol = 0
    for b in range(cfg.nb):
        chunk_table[b] = {k7: [] for k7 in range(cfg.bw)}
        for s in range(NSEG):
            t = 0
            for k7 in range(cfg.bw):
                k = b * cfg.bw + k7
                cell_t0[k, s] = t
                rem = int(t_cell[k, s])
                while rem > 0:
                    col, p0 = t // P, t % P
                    # PE row tiles are 32-granular: [p0, p0+kc) must fit one.
                    if p0 in (32, 96):
                        kc = min(32, rem)
                    elif p0 == 64:
                        kc = min(64, rem)
                    else:
                        kc = min(P, rem)
                    chunk_table[b][k7].append((s, col, p0, kc))
                    t += kc
                    rem -= kc
            tok = (t + P - 1) // P * P
            nb_bs[(b, s)] = tok // P
            tok_bs[(b, s)] = tok
            ic0_bs[(b, s)] = icol
            gcol0_bs[(b, s)] = gcol
            icol += tok // 16
            gcol += tok // P
    ctot, ictot = gcol, icol

    plan = Plan(cfg, t_cell, nb_bs, tok_bs, ic0_bs, gcol0_bs, cell_t0,
                chunk_table, ctot, ictot, core_slots=core_slots)

    for c in range(NC):
        idxs = np.zeros((P, ictot), np.int16)
        dst_all = np.full((P, ctot), -1.0, np.float32)
        for b in range(cfg.nb):
            for s in range(NSEG):
                tok = tok_bs[(b, s)]
                if tok == 0:
                    continue
                strm_i = np.zeros(tok, np.int64)
                strm_d = np.full(tok, -1.0, np.float32)
                base = _seg_base(cfg, s)
                for k7 in range(cfg.bw):
                    k = b * cfg.bw + k7
                    w = core_slots[c][k]
                    ci = c * WPC + k
                    e0 = w_start[ci] + splits[c, k, s]
                    e1 = w_start[ci] + splits[c, k, s + 1]
                    n = e1 - e0
                    t0 = int(cell_t0[k, s])
                    iv = ssrc[e0:e1] - base
                    assert n == 0 or (0 <= iv.min() and iv.max() < 32768), \
                        (c, k, s, int(iv.min()), int(iv.max()))
                    strm_i[t0:t0 + n] = iv
                    strm_d[t0:t0 + n] = (sdst[e0:e1] - w * P).astype(np.float32)
                ic0 = ic0_bs[(b, s)]
                idxs[:, ic0:ic0 + tok // 16] = np.tile(
                    strm_i.astype(np.int16).reshape(-1, 16).T, (8, 1))
                g0 = gcol0_bs[(b, s)]
                dst_all[:, g0:g0 + tok // P] = strm_d.reshape(-1, P).T
        plan.in_maps.append({"idxs": idxs, "dst_all": dst_all})
    return plan


def build_program(cfg: Cfg, plan: Plan):
    from concourse import bacc, mybir
    import concourse.tile as tile

    NSEG, BW, NB, WPC = cfg.n_seg, cfg.bw, cfg.nb, cfg.wpc
    dt = mybir.dt
    no_epi = os.environ.get("GCN_NO_EPI") == "1"
    no_out = os.environ.get("GCN_NO_OUT") == "1"
    no_mm = os.environ.get("GCN_NO_MM") == "1"

    nc = bacc.Bacc("TRN2", target_bir_lowering=False, debug=False,
                   num_devices=cfg.n_cores)

    feat_t = nc.dram_tensor("feature_aug", [cfg.npad, ROW], dt.bfloat16,
                            kind="ExternalInput")
    idxs_t = nc.dram_tensor("idxs", [P, plan.ictot], dt.int16,
                            kind="ExternalInput")
    dstall_t = nc.dram_tensor("dst_all", [P, plan.ctot], dt.float32,
                              kind="ExternalInput")
    normtab_t = nc.dram_tensor("norm_tab", [P, WPC], dt.float32,
                               kind="ExternalInput")
    bb_t = nc.dram_tensor("b_bcast", [P, D], dt.float32, kind="ExternalInput")
    wt_t = nc.dram_tensor("wt", [D, D], dt.bfloat16, kind="ExternalInput")
    iota_t = nc.dram_tensor("iota", [P, P], dt.bfloat16, kind="ExternalInput")
    out_t = nc.dram_tensor("out", [WPC * P, D], dt.float32,
                           kind="ExternalOutput")

    max_nb = max(plan.nb_bs.values())
    max_ic = max(plan.tok_bs[k] // 16 for k in plan.tok_bs)

    with tile.TileContext(nc) as tc:
        with (
            tc.tile_pool(name="const", bufs=1) as cpool,
            tc.tile_pool(name="msgs", bufs=8) as mpool,
            tc.tile_pool(name="idx", bufs=8) as ipool,
            tc.tile_pool(name="w", bufs=8) as wpool,
            tc.tile_pool(name="oh", bufs=6) as ohpool,
            tc.tile_pool(name="acc", bufs=6) as apool,
            tc.tile_pool(name="stage", bufs=2) as spool,
            tc.tile_pool(name="psA", bufs=2, space="PSUM") as psA,
            tc.tile_pool(name="psB", bufs=2, space="PSUM") as psB,
            tc.tile_pool(name="psO", bufs=2, space="PSUM") as psO,
        ):
            iota_s = cpool.tile([P, P], dt.bfloat16, tag="iota")
            nc.sync.dma_start(out=iota_s[:], in_=iota_t[:, :])
            wt_s = cpool.tile([D, D], dt.bfloat16, tag="wt")
            nc.sync.dma_start(out=wt_s[:], in_=wt_t[:, :])
            bb_s = cpool.tile([P, D], dt.float32, tag="bb")
            nc.sync.dma_start(out=bb_s[:], in_=bb_t[:, :])
            ntab_s = cpool.tile([P, WPC], dt.float32, tag="ntab")
            nc.sync.dma_start(out=ntab_s[:], in_=normtab_t[:, :])
            dst_s = cpool.tile([P, plan.ctot], dt.float32, tag="dst")
            nc.sync.dma_start(out=dst_s[:], in_=dstall_t[:, :])

            for b in range(NB):
                msgs = {}
                wts = {}
                for s in range(NSEG):
                    nb = plan.nb_bs[(b, s)]
                    if nb == 0:
                        continue
                    tok = plan.tok_bs[(b, s)]
                    nic = tok // 16
                    it = ipool.tile([P, max_ic], dt.int16, tag="idx")
                    ic0 = plan.ic0_bs[(b, s)]
                    nc.sync.dma_start(out=it[:, :nic],
                                      in_=idxs_t[:, ic0:ic0 + nic])
                    mt = mpool.tile([P, max_nb, ROW], dt.bfloat16, tag="msgs")
                    base = _seg_base(cfg, s)
                    hi = min(base + 32768, cfg.npad)
                    nc.gpsimd.dma_gather(
                        mt[:, :nb, :],
                        feat_t[base:hi, :],
                        it[:, :nic],
                        tok,
                        tok,
                        ROW,
                        single_packet=False,
                    )
                    msgs[s] = mt
                    wtl = wpool.tile([P, max_nb, 1], dt.float32, tag="w")
                    nc.vector.tensor_copy(out=wtl[:, :nb, :],
                                          in_=mt[:, :nb, 64:65])
                    wts[s] = wtl

                ps_a = psA.tile([D, 4 * P], dt.float32, tag="psA")
                ps_b = psB.tile([D, max(BW - 4, 1) * P], dt.float32, tag="psB")
                stage = spool.tile([P, BW * D], dt.float32, tag="stage")

                for k7 in range(BW):
                    k = b * BW + k7
                    if k7 < 4:
                        accT = ps_a[:, k7 * P:(k7 + 1) * P]
                    else:
                        accT = ps_b[:, (k7 - 4) * P:(k7 - 3) * P]
                    chunks = plan.chunk_table[b][k7]
                    for ci, (s, col, p0, kc) in enumerate(chunks):
                        mt, wtl = msgs[s], wts[s]
                        gcol = plan.gcol0_bs[(b, s)] + col
                        oh = ohpool.tile([P, P], dt.bfloat16, tag="oh")
                        nc.vector.tensor_scalar(
                            out=oh[p0:p0 + kc, :],
                            in0=iota_s[p0:p0 + kc, :],
                            scalar1=dst_s[p0:p0 + kc, gcol:gcol + 1],
                            scalar2=wtl[p0:p0 + kc, col, :],
                            op0=mybir.AluOpType.is_equal,
                            op1=mybir.AluOpType.mult,
                        )
                        if no_mm:
                            continue
                        nc.tensor.matmul(
                            out=accT,
                            lhsT=mt[p0:p0 + kc, col, 0:D],
                            rhs=oh[p0:p0 + kc, :],
                            start=(ci == 0),
                            stop=(ci == len(chunks) - 1),
                            tile_position=(p0, 0) if p0 else None,
                        )
                    st_sl = stage[:, k7 * D:(k7 + 1) * D]
                    if no_epi or no_mm:
                        continue
                    if not chunks:
                        nc.vector.tensor_copy(out=st_sl, in_=bb_s[:])
                        continue
                    # PSUM-reading epilogue ops run on the scalar engine
                    # (ACT): DVE PSUM reads concurrent with dma_gather's
                    # SWDGE descriptor generation can wedge the device
                    # (GpSimd<->DVE SBUF port interaction).
                    epi_lvl = int(os.environ.get("GCN_EPI_LVL", "4"))
                    acc_sb = apool.tile([D, P], dt.bfloat16, tag="acc")
                    nc.scalar.copy(out=acc_sb[:], in_=accT)
                    if epi_lvl < 2:
                        continue
                    ops = psO.tile([P, D], dt.float32, tag="psO")
                    nc.tensor.matmul(out=ops[:], lhsT=acc_sb[:], rhs=wt_s[:],
                                     start=True, stop=True)
                    if epi_lvl < 3:
                        continue
                    nc.scalar.activation(
                        out=st_sl, in_=ops[:],
                        func=mybir.ActivationFunctionType.Copy,
                        scale=ntab_s[:, k:k + 1])
                    if epi_lvl < 4:
                        continue
                    nc.vector.tensor_tensor(out=st_sl, in0=st_sl, in1=bb_s[:],
                                            op=mybir.AluOpType.add)
                if no_out or no_epi or no_mm:
                    continue
                ov = out_t[b * BW * P:(b + 1) * BW * P, :]
                ov = ov.rearrange("(kk p) d -> p kk d", p=P)
                nc.sync.dma_start(out=ov, in_=stage[:])

    nc.compile()
    return nc


def host_inputs(cfg: Cfg, plan: Plan, feature, norm, W, b):
    feature = np.asarray(feature, np.float32)
    norm = np.asarray(norm, np.float32).reshape(-1)
    n = feature.shape[0]

    feat_aug = np.zeros((cfg.npad, ROW), BF16)
    feat_aug[:n, :D] = feature.astype(BF16)
    feat_aug[:n, D] = norm.astype(BF16)

    iota = np.tile(np.arange(P, dtype=np.float32), (P, 1)).astype(BF16)
    wt = np.asarray(W, np.float32).T.astype(BF16).copy()  # [din, dout]
    b_bcast = np.tile(np.asarray(b, np.float32), (P, 1)).astype(np.float32)

    in_maps = []
    for c in range(cfg.n_cores):
        ntab = np.zeros((P, cfg.wpc), np.float32)
        for k, w in enumerate(plan.core_slots[c]):
            n0 = w * P
            n1 = min(n0 + P, n)
            if n1 > n0:
                ntab[:n1 - n0, k] = norm[n0:n1]
        m = {
            "feature_aug": feat_aug,
            "idxs": plan.in_maps[c]["idxs"],
            "dst_all": plan.in_maps[c]["dst_all"],
            "norm_tab": ntab,
            "b_bcast": b_bcast,
            "wt": wt,
            "iota": iota,
        }
        in_maps.append(m)
    return in_maps


def assemble_output(cfg: Cfg, plan: Plan, outs, n_nodes):
    full = np.zeros((n_nodes, D), np.float32)
    for c in range(cfg.n_cores):
        oc = outs[c]
        for k, w in enumerate(plan.core_slots[c]):
            n0 = w * P
            n1 = min(n0 + P, n_nodes)
            if n1 > n0:
                full[n0:n1] = oc[k * P:k * P + (n1 - n0)]
    return full


def make_runner(nc, n_cores):
    """Build the sharded jit callable around the compiled Bass program,
    mirroring bass2jax.run_bass_via_pjrt (multi-core branch)."""
    import jax
    from jax.sharding import Mesh, PartitionSpec, NamedSharding
    from jax.experimental.shard_map import shard_map
    from concourse import bass2jax, mybir

    bass2jax.install_neuronx_cc_hook()
    part_name = (nc.partition_id_tensor.name
                 if nc.partition_id_tensor is not None else None)
    in_names, out_names, out_avals, zero_outs = [], [], [], []
    for alloc in nc.m.functions[0].allocations:
        if not isinstance(alloc, mybir.MemoryLocationSet):
            continue
        name = alloc.memorylocations[0].name
        if alloc.kind == "ExternalInput":
            if name == part_name:
                continue
            in_names.append(name)
        elif alloc.kind == "ExternalOutput":
            shape = tuple(alloc.tensor_shape)
            dtype = mybir.dt.np(alloc.dtype)
            out_names.append(name)
            out_avals.append(jax.core.ShapedArray(shape, dtype))
            zero_outs.append(np.zeros(shape, dtype))
    n_params = len(in_names)

    bind_names = in_names + out_names
    if part_name is not None:
        bind_names = bind_names + [part_name]

    def _body(*args):
        operands = list(args)
        if part_name is not None:
            operands.append(bass2jax.partition_id_tensor())
        outs = bass2jax._bass_exec_p.bind(
            *operands,
            out_avals=tuple(out_avals),
            in_names=tuple(bind_names),
            out_names=tuple(out_names),
            lowering_input_output_aliases=(),
            sim_require_finite=True,
            sim_require_nnan=True,
            nc=nc,
        )
        return tuple(outs)

    devices = jax.devices()[:n_cores]
    mesh = Mesh(np.asarray(devices), ("core",))
    spec = PartitionSpec("core")
    n_outs = len(out_names)
    donate = tuple(range(n_params, n_params + n_outs))
    fn = jax.jit(
        shard_map(_body, mesh=mesh, in_specs=(spec,) * (n_params + n_outs),
                  out_specs=(spec,) * n_outs, check_rep=False),
        donate_argnums=donate, keep_unused=True)
    sharding = NamedSharding(mesh, spec)

    class Runner:
        pass

    r = Runner()
    r.fn = fn
    r.in_names = in_names
    r.out_names = out_names
    r.out_avals = out_avals
    r.zero_outs = zero_outs
    r.sharding = sharding
    r.n_cores = n_cores

    def put_inputs(in_maps):
        import jax
        concat = [np.concatenate([np.asarray(m[nm]) for m in in_maps], axis=0)
                  for nm in in_names]
        return [jax.device_put(a, sharding) for a in concat]

    def put_zeros():
        import jax
        return [jax.device_put(
            np.zeros((n_cores * z.shape[0], *z.shape[1:]), z.dtype), sharding)
            for z in zero_outs]

    def run(dev_in, dev_zeros=None):
        import jax
        if dev_zeros is None:
            dev_zeros = put_zeros()
        out = fn(*dev_in, *dev_zeros)
        jax.block_until_ready(out)
        return out

    r.put_inputs = put_inputs
    r.put_zeros = put_zeros
    r.run = run
    return r


_CACHE = {}


def _kernel_device(feature, norm, src, dst, W, b):
    cfg = Cfg()
    feature = np.asarray(feature)
    n = feature.shape[0]
    assert n == cfg.n_nodes, f"unexpected node count {n}"

    plan = make_plan(cfg, src, dst)
    key = plan.t_cell.tobytes()
    if key not in _CACHE:
        nc = build_program(cfg, plan)
        _CACHE[key] = (nc, make_runner(nc, cfg.n_cores))
    nc, runner = _CACHE[key]

    in_maps = host_inputs(cfg, plan, feature, norm, W, b)
    dev_in = runner.put_inputs(in_maps)
    out = None
    last_err = None
    for attempt in range(3):
        try:
            out = runner.run(dev_in)
            break
        except Exception as e:  # transient terminal desyncs: retry
            last_err = e
            import time as _time
            _time.sleep(5.0)
            try:
                dev_in = runner.put_inputs(in_maps)
            except Exception:
                pass
    if out is None:
        raise last_err
    kernel.last_runner = runner
    kernel.last_dev_in = dev_in
    oidx = runner.out_names.index("out")
    shape = runner.out_avals[oidx].shape
    arr = np.asarray(out[oidx]).reshape(cfg.n_cores, *shape)
    outs = [arr[c] for c in range(cfg.n_cores)]
    return assemble_output(cfg, plan, outs, n)


def _worker(work_dir):
    """Subprocess entry: load inputs, run on device, save output."""
    names = ["feature", "norm", "src", "dst", "W", "b"]
    ins = {nm: np.load(os.path.join(work_dir, nm + ".npy")) for nm in names}
    out = _kernel_device(**ins)
    np.save(os.path.join(work_dir, "out.npy"), out)


def kernel(feature, norm, src, dst, W, b):
    """Run the GCN layer on 8 trn2 cores.

    The device work runs in a subprocess: a terminal-side failure
    ("mesh desynced") poisons the whole jax client process, so isolating it
    lets us retry cleanly and keeps the caller's process healthy.
    """
    if os.environ.get("GCN_INPROC") == "1":
        return _kernel_device(feature, norm, src, dst, W, b)

    import subprocess
    import sys
    import tempfile

    kdir = os.path.dirname(os.path.abspath(__file__))
    with tempfile.TemporaryDirectory() as td:
        for nm, arr in [("feature", feature), ("norm", norm), ("src", src),
                        ("dst", dst), ("W", W), ("b", b)]:
            np.save(os.path.join(td, nm + ".npy"), np.asarray(arr))
        code = (f"import sys; sys.path.insert(0, {kdir!r}); "
                f"import kernel; kernel._worker({td!r})")
        last = None
        for attempt in range(4):
            r = subprocess.run([sys.executable, "-c", code],
                               capture_output=True, text=True, timeout=1800)
            if r.returncode == 0 and os.path.exists(
                    os.path.join(td, "out.npy")):
                return np.load(os.path.join(td, "out.npy"))
            last = r.stderr[-2000:] if r.stderr else "unknown"
            import time as _time
            _time.sleep(10.0)
        raise RuntimeError(f"device run failed after retries: {last}")


kernel.last_runner = None
kernel.last_dev_in = None
